# revision 1
# baseline (speedup 1.0000x reference)
"""Trainium2 Bass kernel for nn_MetaPosterior — v4: pipelined Pool+PE hybrid.

v3 (Pool 2048 pairs + PE 2048 pairs) was chunk-serial: each 512-col chunk
walked PE->ACT->DVE->PE->ACT->DVE before the next started, so cross-engine
semaphore latency dominated (30 us/iter, worse than v2's Pool-only 28.5).
v4 fixes that:

- Every pipeline tile (PSUM Y, SBUF yc/z, PSUM u/G, SBUF uu/gg) is double-
  buffered by chunk parity, and chunk semaphores count globally across
  iterations, so stage k of chunk n overlaps stage k+1 of chunk n-1 with
  pipeline depth 2 and no per-iteration barrier.
- The two table fields share ONE stationary: S[r, j] = g-block j (j<64) or
  h'-block j-64 (j>=64).  One gather matmul per (slot, chunk) instead of
  two; one ACT copy per (slot, chunk).  Masks select (Q, field):
  M_g[Q, t], M_h[64+Q, t].  Pairs touching block Q=64 (table rows
  8192..8199, ~8 of 4096) are ineligible and routed to the Pool side by
  the host.
- PSUM: 2x2 Y tiles + 2x2 u/G tiles = exactly 8 banks.

Per chunk: 2 gather-mm + 4 reduce-mm (PE), 2+2 copies (ACT), 4 masks +
2 finals (DVE).  Pool route: ap_gather of the other 2048 pairs
(num_idxs=512, ~14 us) as in v2.
"""

import numpy as np
import ml_dtypes

import concourse.bacc as bacc
import concourse.mybir as mybir
from concourse.bass_utils import run_bass_kernel_spmd

LOG2PI = float(np.log(2.0 * np.pi))
DIM, K, P, M_COND = 1024, 8, 4, 2
D1 = DIM + 1                      # 1025
N_CORES = 8
A_PER_CORE = DIM // N_CORES       # 128
TRIPLES = A_PER_CORE * P * K      # 4096 pairs per core
TBL_ROWS = K * D1                 # 8200 table rows
DF = 2                            # f16 fields per row: [g, h']
CNT_COLS = (TBL_ROWS + 127) // 128  # 65

N_PE = 2048                       # pairs on the PE route
CH = 512                          # PE chunk columns
NCH = N_PE // CH                  # 4 chunks
N_POOL = TRIPLES - N_PE           # pairs on the Pool route
N_GROUPS = 8
PAIRS_G = N_POOL // N_GROUPS      # 256
NIDX_G = 2 * PAIRS_G              # 512 gather idx per group
IDX_COLS = NIDX_G // 16           # 32
RED_COLS = 16

_PROGS = {}


def _build_program(iters=1):
    f32 = mybir.dt.float32
    f16 = mybir.dt.float16
    bf16 = mybir.dt.bfloat16
    i16 = mybir.dt.int16
    alu = mybir.AluOpType
    nc = bacc.Bacc("TRN2", detect_race_conditions=False)

    tbl = nc.dram_tensor("tbl", [128, TBL_ROWS, DF], f16, kind="ExternalInput")
    idx = nc.dram_tensor("idx", [128, IDX_COLS], i16, kind="ExternalInput")
    cnt = nc.dram_tensor("cnt", [128, CNT_COLS], f32, kind="ExternalInput")
    asp = nc.dram_tensor("asp", [128, CNT_COLS], f32, kind="ExternalInput")
    oh = nc.dram_tensor("oh", [128, 2 * N_PE], bf16, kind="ExternalInput")
    mg = nc.dram_tensor("mg", [128, 2 * N_PE], bf16, kind="ExternalInput")
    mh = nc.dram_tensor("mh", [128, 2 * N_PE], bf16, kind="ExternalInput")
    sbo = nc.dram_tensor("sbo", [128, 128], bf16, kind="ExternalInput")
    out = nc.dram_tensor("out", [128, RED_COLS], f32, kind="ExternalOutput")

    import contextlib
    with contextlib.ExitStack() as _es:
        ec = _es.enter_context
        tbl_sb = ec(nc.sbuf_tensor("tbl_sb", [128, TBL_ROWS, DF], f16))
        idx_sb = ec(nc.sbuf_tensor("idx_sb", [128, IDX_COLS], i16))
        cnt_sb = ec(nc.sbuf_tensor("cnt_sb", [128, CNT_COLS], f32))
        asp_sb = ec(nc.sbuf_tensor("asp_sb", [128, CNT_COLS], f32))
        oh_sb = ec(nc.sbuf_tensor("oh_sb", [128, 2 * N_PE], bf16))
        mg_sb = ec(nc.sbuf_tensor("mg_sb", [128, 2 * N_PE], bf16))
        mh_sb = ec(nc.sbuf_tensor("mh_sb", [128, 2 * N_PE], bf16))
        sbo_sb = ec(nc.sbuf_tensor("sbo_sb", [128, 128], bf16))
        ones_sb = ec(nc.sbuf_tensor("ones_sb", [128, 128], bf16))
        gath0 = ec(nc.sbuf_tensor("gath0", [128, NIDX_G, DF], f16))
        gath1 = ec(nc.sbuf_tensor("gath1", [128, NIDX_G, DF], f16))
        gath2 = ec(nc.sbuf_tensor("gath2", [128, NIDX_G, DF], f16))
        u = ec(nc.sbuf_tensor("u", [128, PAIRS_G], f32))
        s2 = ec(nc.sbuf_tensor("s2", [128, PAIRS_G], f32))
        gs = ec(nc.sbuf_tensor("gs", [128, PAIRS_G], f32))
        j1 = ec(nc.sbuf_tensor("j1", [128, PAIRS_G], f32))
        j2 = ec(nc.sbuf_tensor("j2", [128, CNT_COLS], f32))
        # double-buffered pipeline tiles (index = chunk parity)
        yc = [[ec(nc.sbuf_tensor(f"yc{b}{s}", [128, CH], bf16))
               for s in range(2)] for b in range(2)]
        zg = [[ec(nc.sbuf_tensor(f"zg{b}{s}", [128, CH], bf16))
               for s in range(2)] for b in range(2)]
        zh = [[ec(nc.sbuf_tensor(f"zh{b}{s}", [128, CH], bf16))
               for s in range(2)] for b in range(2)]
        uu = [ec(nc.sbuf_tensor(f"uu{b}", [128, CH], f32)) for b in range(2)]
        gg = [ec(nc.sbuf_tensor(f"gg{b}", [128, CH], f32)) for b in range(2)]
        su = ec(nc.sbuf_tensor("su", [128, CH], f32))
        sj = ec(nc.sbuf_tensor("sj", [128, CH], f32))
        red = ec(nc.sbuf_tensor("red", [128, RED_COLS], f32))
        yp = [[ec(nc.psum_tensor(f"yp{b}{s}", [128, CH], f32))
               for s in range(2)] for b in range(2)]
        up = [ec(nc.psum_tensor(f"up{b}", [128, CH], f32)) for b in range(2)]
        gp_ = [ec(nc.psum_tensor(f"gp{b}", [128, CH], f32)) for b in range(2)]
        s_in = ec(nc.semaphore("s_in"))
        s_g = ec(nc.semaphore("s_g"))
        s_v = ec(nc.semaphore("s_v"))
        s_y = ec(nc.semaphore("s_y"))
        s_cp = ec(nc.semaphore("s_cp"))
        s_z = ec(nc.semaphore("s_z"))
        s_red = ec(nc.semaphore("s_red"))
        s_ucp = ec(nc.semaphore("s_ucp"))
        s_fin = ec(nc.semaphore("s_fin"))
        s_o = ec(nc.semaphore("s_o"))
        block = ec(nc.Block())
        bufs = [gath0, gath1, gath2]
        NTOT = iters * NCH

        @block.gpsimd
        def _(gp):
            gp.dma_start(idx_sb[:], idx[:]).then_inc(s_in, 16)
            gp.dma_start(tbl_sb[:], tbl[:]).then_inc(s_in, 16)
            gp.dma_start(cnt_sb[:], cnt[:]).then_inc(s_in, 16)
            gp.dma_start(asp_sb[:], asp[:]).then_inc(s_in, 16)
            gp.dma_start(oh_sb[:], oh[:]).then_inc(s_in, 16)
            gp.dma_start(mg_sb[:], mg[:]).then_inc(s_in, 16)
            gp.dma_start(mh_sb[:], mh[:]).then_inc(s_in, 16)
            gp.dma_start(sbo_sb[:], sbo[:]).then_inc(s_in, 16)
            gp.wait_ge(s_in, 128)
            for i in range(iters):
                if i >= 3:
                    gp.wait_ge(s_v, i - 2)
                gp.ap_gather(
                    bufs[i % 3][:], tbl_sb[:], idx_sb[:],
                    channels=128, num_elems=TBL_ROWS, d=DF, num_idxs=NIDX_G,
                ).then_inc(s_g, 1)

        @block.tensor
        def _(pe):
            pe.wait_ge(s_in, 128)

            def ymm(n):
                b = n % 2
                c = n % NCH
                c0 = c * CH
                c1 = N_PE + c * CH
                if n >= 2:  # Y[b] free once ACT copied chunk n-2
                    pe.wait_ge(s_cp, n - 1)
                pe.matmul(yp[b][0][:], sbo_sb[:], oh_sb[:, c0 : c0 + CH])
                pe.matmul(yp[b][1][:], sbo_sb[:], oh_sb[:, c1 : c1 + CH]) \
                    .then_inc(s_y, 1)

            def redmm(n):
                b = n % 2
                pe.wait_ge(s_z, n + 1)
                if n >= 2:  # u/G psum free once ACT copied chunk n-2
                    pe.wait_ge(s_ucp, n - 1)
                pe.matmul(up[b][:], ones_sb[:], zh[b][0][:],
                          start=True, stop=False)
                pe.matmul(up[b][:], ones_sb[:], zh[b][1][:],
                          start=False, stop=True)
                pe.matmul(gp_[b][:], ones_sb[:], zg[b][0][:],
                          start=True, stop=False)
                pe.matmul(gp_[b][:], ones_sb[:], zg[b][1][:],
                          start=False, stop=True).then_inc(s_red, 1)

            # software pipeline: issue y(n+1) before red(n)
            ymm(0)
            for n in range(NTOT):
                if n + 1 < NTOT:
                    ymm(n + 1)
                redmm(n)

        @block.scalar
        def _(sc):

            def yccp(n):
                b = n % 2
                sc.wait_ge(s_y, n + 1)
                if n >= 2:  # yc[b] free once DVE masked chunk n-2
                    sc.wait_ge(s_z, n - 1)
                sc.copy(yc[b][0][:], yp[b][0][:])
                sc.copy(yc[b][1][:], yp[b][1][:]).then_inc(s_cp, 1)

            def ugcp(n):
                b = n % 2
                sc.wait_ge(s_red, n + 1)
                if n >= 2:  # uu/gg[b] free once DVE finals chunk n-2
                    sc.wait_ge(s_fin, n - 1)
                sc.copy(uu[b][:], up[b][:])
                sc.copy(gg[b][:], gp_[b][:]).then_inc(s_ucp, 1)

            yccp(0)
            for n in range(NTOT):
                if n + 1 < NTOT:
                    yccp(n + 1)
                ugcp(n)

        @block.vector
        def _(v):
            v.memset(ones_sb[:], 1.0)
            v.memset(red[:], 0.0)
            def masks(n):
                b = n % 2
                c = n % NCH
                c0 = c * CH
                c1 = N_PE + c * CH
                v.wait_ge(s_cp, n + 1)
                if n >= 2:  # z[b] free once PE reduced chunk n-2
                    v.wait_ge(s_red, n - 1)
                v.tensor_tensor(zg[b][0][:], yc[b][0][:],
                                mg_sb[:, c0 : c0 + CH], alu.mult)
                v.tensor_tensor(zg[b][1][:], yc[b][1][:],
                                mg_sb[:, c1 : c1 + CH], alu.mult)
                v.tensor_tensor(zh[b][0][:], yc[b][0][:],
                                mh_sb[:, c0 : c0 + CH], alu.mult)
                v.tensor_tensor(zh[b][1][:], yc[b][1][:],
                                mh_sb[:, c1 : c1 + CH], alu.mult) \
                    .then_inc(s_z, 1)

            def finals(n):
                b = n % 2
                c = n % NCH
                v.wait_ge(s_ucp, n + 1)
                v.scalar_tensor_tensor(
                    su[:], uu[b][:], 0.0, uu[b][:], alu.add, alu.mult,
                    accum_out=red[:, 4 + c : 5 + c],
                )
                v.scalar_tensor_tensor(
                    sj[:], gg[b][:], -0.5, su[:], alu.mult, alu.mult,
                    accum_out=red[:, 8 + c : 9 + c],
                ).then_inc(s_fin, 1)

            masks(0)
            for i in range(iters):
                for c in range(NCH):
                    n = i * NCH + c
                    if n + 1 < NTOT:
                        masks(n + 1)
                    finals(n)
                # pool-route ops
                g = bufs[i % 3]
                g0 = g[:, 0:PAIRS_G, 0]
                g1 = g[:, PAIRS_G:NIDX_G, 0]
                h0 = g[:, 0:PAIRS_G, 1]
                h1 = g[:, PAIRS_G:NIDX_G, 1]
                v.wait_ge(s_g, i + 1)
                v.tensor_tensor(u[:], h0, h1, alu.add)
                v.scalar_tensor_tensor(
                    s2[:], u[:], 0.0, u[:], alu.add, alu.mult,
                    accum_out=red[:, 0:1],
                )
                v.tensor_tensor(gs[:], g0, g1, alu.add)
                v.scalar_tensor_tensor(
                    j1[:], gs[:], -0.5, s2[:], alu.mult, alu.mult,
                    accum_out=red[:, 1:2],
                )
                v.scalar_tensor_tensor(
                    j2[:], cnt_sb[:], 1.0, asp_sb[:], alu.mult, alu.mult,
                    accum_out=red[:, 2:3],
                ).then_inc(s_v, 1)

        @block.sync
        def _(s):
            s.wait_ge(s_v, iters)
            s.wait_ge(s_fin, NTOT)
            s.dma_start(out[:], red[:]).then_inc(s_o, 16)
            s.wait_ge(s_o, 16)

    nc.finalize()
    return nc


def _get_program(iters=1):
    if iters not in _PROGS:
        _PROGS[iters] = _build_program(iters)
    return _PROGS[iters]


def _make_tables(meta_theta, m_ks, grads_v):
    g = np.asarray(grads_v, np.float64)
    c = np.asarray(meta_theta, np.float64)[None, :] - np.asarray(m_ks, np.float64)
    h = g * c
    t_k = h.sum(axis=1)
    a_f = 0.5 * np.log(g) - 0.5 * g * c * c
    hp = h - 0.5 * t_k[:, None]
    ap = a_f + t_k[:, None] * h - 0.5 * (t_k * t_k)[:, None]

    tbl = np.empty((TBL_ROWS, DF), np.float16)
    tbl[:, 0] = g.ravel().astype(np.float16)
    tbl[:, 1] = hp.ravel().astype(np.float16)

    asp = np.zeros(128 * CNT_COLS, np.float64)
    asp[:TBL_ROWS] = ap.ravel()
    asp = np.ascontiguousarray(asp.reshape(CNT_COLS, 128).T.astype(np.float32))

    # combined stationary: col j<64 = g block j, col j>=64 = h' block j-64
    bf = ml_dtypes.bfloat16
    sbo = np.zeros((128, 128), bf)
    sbo[:, 0:64] = g.ravel()[: 64 * 128].reshape(64, 128).T.astype(bf)
    sbo[:, 64:128] = hp.ravel()[: 64 * 128].reshape(64, 128).T.astype(bf)
    return tbl, asp, sbo


def _device_inputs(meta_theta, m_ks, grads_v, perms):
    tbl, asp, sbo = _make_tables(meta_theta, m_ks, grads_v)
    tbl_rep = np.ascontiguousarray(np.broadcast_to(tbl[None], (128, TBL_ROWS, DF)))
    bf = ml_dtypes.bfloat16

    perms01 = np.ascontiguousarray(np.asarray(perms)[:, :, :, :2])
    kvec = np.tile(np.arange(K, dtype=np.int64), TRIPLES // K)

    in_maps = []
    for core in range(N_CORES):
        sl = perms01[core * A_PER_CORE : (core + 1) * A_PER_CORE]
        sl = sl.reshape(TRIPLES, 2).astype(np.int64)
        comb = kvec[:, None] * D1 + sl                    # (4096, 2)

        cntv = np.bincount(comb.ravel(), minlength=128 * CNT_COLS)
        cnt = np.ascontiguousarray(
            cntv.reshape(CNT_COLS, 128).T.astype(np.float32)
        )

        # PE route: first N_PE pairs with both slots in blocks Q < 64
        q_all = comb // 128
        elig = (q_all[:, 0] < 64) & (q_all[:, 1] < 64)
        pe_sel = np.where(elig)[0]
        assert len(pe_sel) >= N_PE, len(pe_sel)
        pe_sel = pe_sel[:N_PE]
        pool_sel = np.setdiff1d(np.arange(TRIPLES), pe_sel)

        cpe = comb[pe_sel]                                # (N_PE, 2)
        r = (cpe % 128).astype(np.int64)
        q = (cpe // 128).astype(np.int64)
        oh = np.zeros((128, 2 * N_PE), bf)
        mg = np.zeros((128, 2 * N_PE), bf)
        mh = np.zeros((128, 2 * N_PE), bf)
        t_ar = np.arange(N_PE)
        for s in range(2):
            oh[r[:, s], s * N_PE + t_ar] = 1
            mg[q[:, s], s * N_PE + t_ar] = 1
            mh[64 + q[:, s], s * N_PE + t_ar] = 1

        cpool = comb[pool_sel].reshape(N_GROUPS, PAIRS_G, 2)
        idxg = np.empty((N_GROUPS, NIDX_G), np.int16)
        idxg[:, :PAIRS_G] = cpool[:, :, 0]
        idxg[:, PAIRS_G:] = cpool[:, :, 1]
        idx128 = idxg.reshape(N_GROUPS, IDX_COLS, 16).transpose(0, 2, 1)
        idx128 = np.ascontiguousarray(idx128.reshape(128, IDX_COLS))

        in_maps.append({
            "tbl": tbl_rep, "idx": idx128, "cnt": cnt, "asp": asp,
            "oh": oh, "mg": mg, "mh": mh, "sbo": sbo,
        })
    return in_maps


def _finalize(partials, meta_theta, alpha):
    p = np.asarray(partials, np.float64)
    total = float(
        p[:, ::16, 0:2].sum()
        + p[:, :, 2].sum()
        + p[:, 0, 4 : 4 + NCH].sum()
        + p[:, 0, 8 : 8 + NCH].sum()
    )
    sum_lp = total - LOG2PI * (N_CORES * TRIPLES)
    loss_pred = sum_lp / (P * M_COND * K)
    mt = np.asarray(meta_theta, np.float64)
    a = float(alpha)
    lp_prior = -0.5 * (D1 * LOG2PI + D1 * np.log(a) + float(mt @ mt) / a)
    loss = (1.0 - 1.0 / K) * lp_prior + loss_pred
    return np.float32(-loss)


def run_device(in_maps, iters=1, **kwargs):
    nc = _get_program(iters)
    return run_bass_kernel_spmd(nc, in_maps, list(range(N_CORES)), **kwargs)


def kernel(meta_theta, m_ks, grads_v, perms, alpha):
    in_maps = _device_inputs(meta_theta, m_ks, grads_v, perms)
    last_err = None
    for _ in range(3):
        try:
            res = run_device(in_maps)
            break
        except Exception as e:  # noqa: BLE001
            last_err = e
    else:
        raise last_err
    partials = np.stack([r["out"] for r in res.results])
    return _finalize(partials, meta_theta, alpha)

